# revision 1
# baseline (speedup 1.0000x reference)
"""Trainium2 Bass kernel for nn_ClockAwareGNN (segment_reduce).

Model (reference, fp32):
    gp   = segment_mean(x, batch) @ W_base + b_base            # [B, 1]
    h    = relu(clock @ W1 + b1) @ W2 + b2                     # [N, 16]
    cp   = segment_mean(h, batch)                              # [B, 16]
    out  = relu([gp | cp] @ W3 + b3) @ W4 + b4                 # [B, 1]

Everything after the segment reductions is affine in per-graph quantities, so
the heavy per-node work collapses to three fused segment reductions:
    Sx[g]  = sum of x rows in graph g          (128 cols)
    Sr[g]  = sum of r rows in graph g          (R cols)
    cnt[g] = node count of graph g
where r is either the raw clock (R=1; exact when b1 == 0 and clock >= 0 since
relu(c*W1) == c*relu(W1) elementwise for c >= 0) or the host-computed
relu(clock @ W1 + b1) (R=16 fallback).

Device strategy (per core, 8-way data-parallel by graph):
  - nodes arrive as 128-row tiles; batch ids are sorted so each tile touches
    <= 2 graphs inside one 32-graph "window".
  - DVE builds one-hot assign tiles [128 nodes, 32 graphs] for a whole
    super-tile in one is_equal op (broadcast AP vs an iota pattern).
  - PE accumulates assign.T @ payload into PSUM [128 graphs, C1+129] fp32.
    x is shipped as bf16 hi (+ r hi/lo) in one matmul and a 512-scaled
    fp8e4m3 lo-correction (+ a ones column providing counts) in a second
    matmul — 3 B/element of x traffic with ~2^-13 effective precision.
  - tiny vector-engine epilogue computes the folded per-graph MLP.
"""

import math
import sys
import types

import numpy as np
import ml_dtypes

import concourse.bass as bass
import concourse.bacc as bacc
import concourse.tile as tile
from concourse import mybir
from concourse.bass_utils import run_bass_kernel_spmd


def _ensure_axon_hooks():
    """bass_utils' trace path does `from antenv.axon_hooks import ...`;
    some agent images lack that submodule. Install it (with the real NTFF
    hook when available) so trace=True degrades gracefully instead of
    raising ModuleNotFoundError."""
    try:
        import antenv  # noqa: F401
        import antenv.axon_hooks  # noqa: F401
        return
    except ImportError:
        pass
    try:
        import antenv
    except ImportError:
        return
    mod = types.ModuleType("antenv.axon_hooks")
    state = {"hook": None}
    mod.set_axon_ntff_profile_hook = lambda h: state.__setitem__("hook", h)
    mod.get_axon_ntff_profile_hook = lambda: state["hook"]
    sys.modules["antenv.axon_hooks"] = mod
    antenv.axon_hooks = mod
    try:
        from trn_agent_boot.trn_boot import _ntff_profile_via_ctypes
        mod.set_axon_ntff_profile_hook(
            _ntff_profile_via_ctypes("/opt/axon/libaxon_pjrt.so"))
    except Exception:
        pass
    # the trace path also uploads the NEFF dir to a bucket; in zero-egress
    # containers that raises — fall back to the local path.
    try:
        import concourse.bass_utils as _bu
        _orig_upload = _bu.upload_artifacts

        def _safe_upload(tmpdir):
            try:
                return _orig_upload(tmpdir)
            except Exception:
                return str(tmpdir)

        _bu.upload_artifacts = _safe_upload
    except Exception:
        pass


_ensure_axon_hooks()

BF16 = ml_dtypes.bfloat16
F8 = ml_dtypes.float8_e4m3

N_CORES = 8
N_GRAPHS = 1024
D = 128                 # feature dim of x
GPC = N_GRAPHS // N_CORES   # graphs per core = 128
W = 32                  # one-hot window width (PSUM partition alignment unit)
WPC = GPC // W          # windows per core = 4
ST = 32                 # node-tiles per DMA super-tile
LO_SCALE = 512.0        # fp8 lo-correction pre-scale (2^9)


def _build_program(S, C1, R, mixed_lhsT):
    """Build the SPMD Bass/Tile program. Shapes are static; per-core data
    differences live entirely in the input tensors.

    S:  number of super-tiles (each ST node-tiles of 128 nodes)
    C1: bf16 payload column count = 128 + 2*R
    mixed_lhsT: if True, the fp8 lo matmul reuses the bf16 assign tile
    """
    fp32 = mybir.dt.float32
    bf16 = mybir.dt.bfloat16
    f8 = mybir.dt.float8e4
    n_tiles = S * ST
    T_w = n_tiles // WPC
    DL = D + 1             # fp8 lo block width: lo cols + count-of-ones col
    C_ps = C1 + DL         # psum cols: bf16 block + lo block

    nc = bacc.Bacc("TRN2", target_bir_lowering=False, debug=False,
                   num_devices=N_CORES)

    xcc = nc.dram_tensor("xcc", [S, 128, ST * C1], bf16, kind="ExternalInput").ap()
    xlo = nc.dram_tensor("xlo", [S, 128, ST * DL], f8, kind="ExternalInput").ap()
    brs = nc.dram_tensor("brs", [128, S * ST], bf16, kind="ExternalInput").ap()
    iota_c = nc.dram_tensor("iota_c", [128, ST * W], bf16, kind="ExternalInput").ap()
    wbase_b = nc.dram_tensor("wbase_b", [128, D], fp32, kind="ExternalInput").ap()
    v1_b = nc.dram_tensor("v1_b", [128, 32], fp32, kind="ExternalInput").ap()
    m2_b = nc.dram_tensor("m2_b", [128, R * 32], fp32, kind="ExternalInput").ap()
    v0_b = nc.dram_tensor("v0_b", [128, 32], fp32, kind="ExternalInput").ap()
    w4_b = nc.dram_tensor("w4_b", [128, 32], fp32, kind="ExternalInput").ap()
    bb_t = nc.dram_tensor("bb_t", [128, 1], fp32, kind="ExternalInput").ap()
    b4_t = nc.dram_tensor("b4_t", [128, 1], fp32, kind="ExternalInput").ap()
    out_d = nc.dram_tensor("out", [128, 1], fp32, kind="ExternalOutput").ap()

    with tile.TileContext(nc) as tc:
        with (
            tc.tile_pool(name="consts", bufs=1) as cpool,
            tc.tile_pool(name="xin", bufs=8) as xpool,
            tc.tile_pool(name="loin", bufs=8) as lpool,
            tc.tile_pool(name="assign", bufs=3) as apool,
            tc.tile_pool(name="epi", bufs=1) as epool,
            tc.tile_pool(name="ps", bufs=1, space="PSUM") as ppool,
        ):
            # ---- constants ----
            iota_t = cpool.tile([128, ST * W], bf16, tag="iota")
            nc.sync.dma_start(iota_t[:], iota_c)
            # whole-run batch-rel ids: one small DMA instead of one per super
            brall = cpool.tile([128, S * ST], bf16, tag="brall")
            nc.sync.dma_start(brall[:], brs)
            wb_t = cpool.tile([128, D], fp32, tag="wb")
            nc.sync.dma_start(wb_t[:], wbase_b)
            v1_t = cpool.tile([128, 32], fp32, tag="v1")
            nc.sync.dma_start(v1_t[:], v1_b)
            m2_t = cpool.tile([128, R * 32], fp32, tag="m2")
            nc.sync.dma_start(m2_t[:], m2_b)
            v0_t = cpool.tile([128, 32], fp32, tag="v0")
            nc.sync.dma_start(v0_t[:], v0_b)
            w4_t = cpool.tile([128, 32], fp32, tag="w4")
            nc.sync.dma_start(w4_t[:], w4_b)
            bbt = cpool.tile([128, 1], fp32, tag="bb")
            nc.sync.dma_start(bbt[:], bb_t)
            b4t = cpool.tile([128, 1], fp32, tag="b4")
            nc.sync.dma_start(b4t[:], b4_t)

            psum = ppool.tile([128, C_ps], fp32, tag="acc")

            # init matmul: zero weights x zero rhs, start=True claims the
            # whole bank's has_written bits so all later matmuls (start=False)
            # overwrite-on-first-touch / accumulate-after, independent of
            # window interleaving.
            zw = cpool.tile([128, 128], bf16, tag="zw")
            nc.vector.memset(zw[:], 0.0)
            zr = cpool.tile([128, C_ps], bf16, tag="zr")
            nc.vector.memset(zr[:], 0.0)
            nc.tensor.matmul(psum[:, :], zw[:], zr[:], start=True, stop=False)

            # ---- main loop ----
            for s in range(S):
                xt = xpool.tile([128, ST * C1], bf16, tag="xt")
                nc.sync.dma_start(xt[:], xcc[s])
                # second HWDGE ring (ACT) for the fp8 lo + batch-rel streams
                lt = lpool.tile([128, ST * DL], f8, tag="lt")
                nc.scalar.dma_start(lt[:], xlo[s])
                # one-hot assign for all ST tiles in one DVE op:
                # asg[p, t, j] = (iota[j] == br[p, s*ST + t])
                asg = apool.tile([128, ST * W], bf16, tag="asg")
                nc.vector.tensor_tensor(
                    asg[:].rearrange("p (t j) -> p t j", j=W),
                    iota_t[:].rearrange("p (t j) -> p t j", j=W),
                    brall[:, s * ST : (s + 1) * ST]
                        .rearrange("p (t o) -> p t o", o=1)
                        .to_broadcast((128, ST, W)),
                    op=mybir.AluOpType.is_equal,
                )
                if mixed_lhsT:
                    asg8 = asg
                else:
                    asg8 = apool.tile([128, ST * W], f8, tag="asg8")
                    nc.vector.tensor_copy(asg8[:], asg[:])
                for t in range(ST):
                    i = s * ST + t
                    w = i // T_w
                    last = i == n_tiles - 1
                    nc.tensor.matmul(
                        psum[w * W : (w + 1) * W, 0:C1],
                        asg[:, t * W : (t + 1) * W],
                        xt[:, t * C1 : (t + 1) * C1],
                        start=False,
                        stop=False,
                        tile_position=(0, w * W),
                    )
                    nc.tensor.matmul(
                        psum[w * W : (w + 1) * W, C1 : C1 + DL],
                        asg8[:, t * W : (t + 1) * W],
                        lt[:, t * DL : (t + 1) * DL],
                        start=False,
                        stop=last,
                        tile_position=(0, w * W),
                    )

            # ---- epilogue (per-graph folded MLP) ----
            sb = epool.tile([128, C_ps], fp32, tag="sb")
            nc.vector.tensor_copy(sb[:], psum[:])

            # Sx = hi_sums + lo_sums / LO_SCALE
            slo = epool.tile([128, D], fp32, tag="slo")
            nc.vector.tensor_scalar_mul(slo[:], sb[:, C1 : C1 + D], 1.0 / LO_SCALE)
            sx = epool.tile([128, D], fp32, tag="sx")
            nc.vector.tensor_add(sx[:], sb[:, 0:D], slo[:])

            sr = epool.tile([128, R], fp32, tag="sr")
            nc.vector.tensor_add(
                sr[:], sb[:, D : D + R], sb[:, D + R : D + 2 * R]
            )
            cntc = epool.tile([128, 1], fp32, tag="cnt")
            nc.vector.tensor_scalar_max(cntc[:], sb[:, C1 + D : C1 + D + 1], 1.0)
            rec = epool.tile([128, 1], fp32, tag="rec")
            nc.vector.reciprocal(rec[:], cntc[:])

            mx = epool.tile([128, D], fp32, tag="mx")
            nc.vector.tensor_scalar_mul(mx[:], sx[:], rec[:])
            mr = epool.tile([128, R], fp32, tag="mr")
            nc.vector.tensor_scalar_mul(mr[:], sr[:], rec[:])

            # gp = rowsum(mean_x * W_base) + b_base
            t1 = epool.tile([128, D], fp32, tag="t1")
            nc.vector.tensor_mul(t1[:], mx[:], wb_t[:])
            gp = epool.tile([128, 1], fp32, tag="gp")
            nc.vector.tensor_reduce(gp[:], t1[:], axis=mybir.AxisListType.X,
                                    op=mybir.AluOpType.add)
            nc.vector.tensor_add(gp[:], gp[:], bbt[:])

            # pre = gp*v1 + sum_j mr[:,j]*M2[j] + v0
            pre = epool.tile([128, 32], fp32, tag="pre")
            nc.vector.tensor_scalar_mul(pre[:], v1_t[:], gp[:])
            tmp = epool.tile([128, 32], fp32, tag="tmp")
            for j in range(R):
                nc.vector.tensor_scalar(
                    tmp[:], m2_t[:, j * 32 : (j + 1) * 32], mr[:, j : j + 1], None,
                    op0=mybir.AluOpType.mult,
                )
                nc.vector.tensor_add(pre[:], pre[:], tmp[:])
            nc.vector.tensor_add(pre[:], pre[:], v0_t[:])

            act = epool.tile([128, 32], fp32, tag="act")
            nc.scalar.activation(act[:], pre[:], mybir.ActivationFunctionType.Relu)

            # out = rowsum(act * W4) + b4
            nc.vector.tensor_mul(act[:], act[:], w4_t[:])
            oo = epool.tile([128, 1], fp32, tag="oo")
            nc.vector.tensor_reduce(oo[:], act[:], axis=mybir.AxisListType.X,
                                    op=mybir.AluOpType.add)
            nc.vector.tensor_add(oo[:], oo[:], b4t[:])

            nc.sync.dma_start(out_d, oo[:])

    nc.compile()
    return nc


def kernel(x, clock_period, batch, W_base, b_base, W1, b1, W2, b2, W3, b3, W4, b4,
           _profile=None, _mixed_lhsT=True):
    x = np.asarray(x, np.float32)
    clock = np.asarray(clock_period, np.float32).reshape(-1)
    batch = np.asarray(batch, np.int32)
    W_base = np.asarray(W_base, np.float32)
    W1 = np.asarray(W1, np.float32); b1 = np.asarray(b1, np.float32)
    W2 = np.asarray(W2, np.float32); b2 = np.asarray(b2, np.float32)
    W3 = np.asarray(W3, np.float32); b3 = np.asarray(b3, np.float32)
    W4 = np.asarray(W4, np.float32); b4 = np.asarray(b4, np.float32)
    hid = W1.shape[1]

    # r-path: exact algebraic fold when relu(c*W1 + b1) == c * relu(W1)
    fold = bool(np.all(b1 == 0.0)) and bool(clock.min() >= 0.0)
    if fold:
        R = 1
        r32 = clock[:, None]                                   # [N, 1]
        q = np.maximum(W1, 0.0) @ W2                           # [1, hid]
        M2 = q @ W3[1:, :]                                     # [1, 32]
        v0 = b2 @ W3[1:, :] + b3                               # [32]
    else:
        R = hid
        r32 = np.maximum(clock[:, None] @ W1 + b1, 0.0)        # [N, hid]
        M2 = W2 @ W3[1:, :]                                    # [hid, 32]
        v0 = b2 @ W3[1:, :] + b3

    C1 = D + 2 * R          # [xhi | rhi | rlo]; count rides in the fp8 block
    assert C1 % 2 == 0
    DL = D + 1

    # ---- shard by graph; window padding so tile->window map is static ----
    cut = np.searchsorted(batch, np.arange(0, N_GRAPHS + 1, W))
    win_nodes = np.diff(cut)
    T_w = int(math.ceil(win_nodes.max() / 128.0))
    while (WPC * T_w) % ST:
        T_w += 1
    n_tiles = WPC * T_w
    S = n_tiles // ST
    Npad = n_tiles * 128

    xhi = x.astype(BF16)
    xlo8 = ((x - xhi.astype(np.float32)) * LO_SCALE).astype(F8)
    rhi = r32.astype(BF16)
    rlo = (r32 - rhi.astype(np.float32)).astype(BF16)

    in_maps = []
    # shared constant tiles
    iota_c = np.broadcast_to(
        np.tile(np.arange(W, dtype=BF16), ST)[None, :], (128, ST * W)
    ).copy()
    wbase_b = np.broadcast_to(W_base[:, 0][None, :], (128, D)).astype(np.float32).copy()
    v1_b = np.broadcast_to(W3[0, :][None, :], (128, 32)).astype(np.float32).copy()
    m2_b = np.broadcast_to(M2.reshape(-1)[None, :], (128, R * 32)).astype(np.float32).copy()
    v0_b = np.broadcast_to(v0[None, :], (128, 32)).astype(np.float32).copy()
    w4_b = np.broadcast_to(W4[:, 0][None, :], (128, 32)).astype(np.float32).copy()
    bb_t = np.full((128, 1), float(b_base.reshape(-1)[0]), np.float32)
    b4_t = np.full((128, 1), float(b4.reshape(-1)[0]), np.float32)

    for k in range(N_CORES):
        xcc = np.zeros((Npad, C1), BF16)
        xl = np.zeros((Npad, DL), F8)
        br = np.full(Npad, -1.0, BF16)
        for wi in range(WPC):
            gw = k * WPC + wi          # global window index
            s0, e0 = int(cut[gw]), int(cut[gw + 1])
            n = e0 - s0
            o = wi * T_w * 128
            xcc[o : o + n, 0:D] = xhi[s0:e0]
            xcc[o : o + n, D : D + R] = rhi[s0:e0]
            xcc[o : o + n, D + R : D + 2 * R] = rlo[s0:e0]
            xl[o : o + n, 0:D] = xlo8[s0:e0]
            xl[o : o + n, D] = F8(1.0)
            br[o : o + n] = (batch[s0:e0] - gw * W).astype(BF16)
        brs = np.ascontiguousarray(br.reshape(S * ST, 128).T)
        # permute so each SBUF partition line is contiguous in DRAM
        xcc_p = np.ascontiguousarray(
            xcc.reshape(S, ST, 128, C1).transpose(0, 2, 1, 3)
        ).reshape(S, 128, ST * C1)
        xlo_p = np.ascontiguousarray(
            xl.reshape(S, ST, 128, DL).transpose(0, 2, 1, 3)
        ).reshape(S, 128, ST * DL)
        in_maps.append(dict(
            xcc=xcc_p, xlo=xlo_p, brs=brs, iota_c=iota_c,
            wbase_b=wbase_b, v1_b=v1_b, m2_b=m2_b, v0_b=v0_b, w4_b=w4_b,
            bb_t=bb_t, b4_t=b4_t,
        ))

    nc = _build_program(S, C1, R, _mixed_lhsT)

    kw = {}
    if _profile is not None:
        kw = dict(trace=True, **_profile)
    res = run_bass_kernel_spmd(nc, in_maps, list(range(N_CORES)), **kw)

    out = np.concatenate([res.results[k]["out"] for k in range(N_CORES)], axis=0)
    if _profile is not None:
        return out.astype(np.float32), res
    return out.astype(np.float32)



# revision 2
# speedup vs baseline: 2.2663x; 2.2663x over previous
"""Trainium2 Bass kernel for nn_ClockAwareGNN (segment_reduce).

Model (reference, fp32):
    gp   = segment_mean(x, batch) @ W_base + b_base            # [B, 1]
    h    = relu(clock @ W1 + b1) @ W2 + b2                     # [N, 16]
    cp   = segment_mean(h, batch)                              # [B, 16]
    out  = relu([gp | cp] @ W3 + b3) @ W4 + b4                 # [B, 1]

Everything after the segment reductions is affine in per-graph quantities, so
the heavy per-node work collapses to fused segment sums:
    Sx[g]  = sum of x rows in graph g          (128 cols)
    Sr[g]  = sum of r rows in graph g          (R cols)
where r is the raw clock (R=1; exact when b1 == 0 and clock >= 0 since
relu(c*W1) == c*relu(W1) elementwise for c >= 0) or relu(clock @ W1 + b1)
(R=hid fallback). Counts are host-side bincounts (shipped as 1/cnt).

Device strategy (per core, 8-way data-parallel by graph):
  - the whole payload [x | r] ships as ONE fp8(e4m3) stream: 129 B/node.
    End-to-end quantization error is ~2.5e-3 of output scale (measured on
    the real input distribution) vs the 2e-2 gate.
  - nodes arrive as 128-row tiles; batch ids are sorted so each tile touches
    <= 2 graphs inside one 32-graph "window" (128 graphs/core = 4 windows).
  - DVE builds one-hot assign tiles [128 nodes, 32 graphs] for a whole
    super-tile in one is_equal op (broadcast AP vs an iota pattern).
  - PE accumulates assign.T @ payload into PSUM [128 graphs, 129] fp32.
    Tiles are interleaved across the 4 windows (tile i -> window i%4) so
    consecutive matmuls land in different PSUM col-groups and overlap in
    the array (tile_position col packing).
  - tiny vector-engine epilogue computes the folded per-graph MLP.
"""

import math
import sys
import types

import numpy as np
import ml_dtypes

import concourse.bass as bass
import concourse.bacc as bacc
import concourse.tile as tile
from concourse import mybir
from concourse.bass_utils import run_bass_kernel_spmd


def _ensure_axon_hooks():
    """bass_utils' trace path does `from antenv.axon_hooks import ...`;
    some agent images lack that submodule. Install it (with the real NTFF
    hook when available) so trace=True degrades gracefully instead of
    raising ModuleNotFoundError."""
    try:
        import antenv  # noqa: F401
        import antenv.axon_hooks  # noqa: F401
        return
    except ImportError:
        pass
    try:
        import antenv
    except ImportError:
        return
    mod = types.ModuleType("antenv.axon_hooks")
    state = {"hook": None}
    mod.set_axon_ntff_profile_hook = lambda h: state.__setitem__("hook", h)
    mod.get_axon_ntff_profile_hook = lambda: state["hook"]
    sys.modules["antenv.axon_hooks"] = mod
    antenv.axon_hooks = mod
    try:
        from trn_agent_boot.trn_boot import _ntff_profile_via_ctypes
        mod.set_axon_ntff_profile_hook(
            _ntff_profile_via_ctypes("/opt/axon/libaxon_pjrt.so"))
    except Exception:
        pass
    # the trace path also uploads the NEFF dir to a bucket; in zero-egress
    # containers that raises — fall back to the local path.
    try:
        import concourse.bass_utils as _bu
        _orig_upload = _bu.upload_artifacts

        def _safe_upload(tmpdir):
            try:
                return _orig_upload(tmpdir)
            except Exception:
                return str(tmpdir)

        _bu.upload_artifacts = _safe_upload
    except Exception:
        pass


_ensure_axon_hooks()

BF16 = ml_dtypes.bfloat16
F8 = ml_dtypes.float8_e4m3

N_CORES = 8
N_GRAPHS = 1024
D = 128                 # feature dim of x
GPC = N_GRAPHS // N_CORES   # graphs per core = 128
W = 32                  # one-hot window width (PSUM partition alignment unit)
WPC = GPC // W          # windows per core = 4
ST = 64                 # node-tiles per DMA super-tile


def _build_program(S, ST_, C, R):
    """Build the SPMD Bass/Tile program. Shapes are static; per-core data
    differences live entirely in the input tensors.

    S:  number of super-tiles (each ST_ node-tiles of 128 nodes)
    C:  fp8 payload column count = 128 + R
    """
    fp32 = mybir.dt.float32
    bf16 = mybir.dt.bfloat16
    f8 = mybir.dt.float8e4
    n_tiles = S * ST_

    nc = bacc.Bacc("TRN2", target_bir_lowering=False, debug=False,
                   num_devices=N_CORES)

    xcc = nc.dram_tensor("xcc", [S, 128, ST_ * C], f8, kind="ExternalInput").ap()
    brs = nc.dram_tensor("brs", [128, n_tiles], bf16, kind="ExternalInput").ap()
    iota_c = nc.dram_tensor("iota_c", [128, ST_ * W], bf16, kind="ExternalInput").ap()
    rec_d = nc.dram_tensor("rec_d", [128, 1], fp32, kind="ExternalInput").ap()
    wbase_b = nc.dram_tensor("wbase_b", [128, D], fp32, kind="ExternalInput").ap()
    v1_b = nc.dram_tensor("v1_b", [128, 32], fp32, kind="ExternalInput").ap()
    m2_b = nc.dram_tensor("m2_b", [128, R * 32], fp32, kind="ExternalInput").ap()
    v0_b = nc.dram_tensor("v0_b", [128, 32], fp32, kind="ExternalInput").ap()
    w4_b = nc.dram_tensor("w4_b", [128, 32], fp32, kind="ExternalInput").ap()
    bb_t = nc.dram_tensor("bb_t", [128, 1], fp32, kind="ExternalInput").ap()
    b4_t = nc.dram_tensor("b4_t", [128, 1], fp32, kind="ExternalInput").ap()
    out_d = nc.dram_tensor("out", [128, 1], fp32, kind="ExternalOutput").ap()

    with tile.TileContext(nc) as tc:
        with (
            tc.tile_pool(name="consts", bufs=1) as cpool,
            tc.tile_pool(name="xin", bufs=6) as xpool,
            tc.tile_pool(name="assign", bufs=3) as apool,
            tc.tile_pool(name="epi", bufs=1) as epool,
            tc.tile_pool(name="ps", bufs=1, space="PSUM") as ppool,
        ):
            # ---- constants ----
            iota_t = cpool.tile([128, ST_ * W], bf16, tag="iota")
            nc.sync.dma_start(iota_t[:], iota_c)
            # whole-run batch-rel ids: one small DMA instead of one per super
            brall = cpool.tile([128, n_tiles], bf16, tag="brall")
            nc.sync.dma_start(brall[:], brs)
            rec_t = cpool.tile([128, 1], fp32, tag="rec")
            nc.scalar.dma_start(rec_t[:], rec_d)
            wb_t = cpool.tile([128, D], fp32, tag="wb")
            nc.scalar.dma_start(wb_t[:], wbase_b)
            v1_t = cpool.tile([128, 32], fp32, tag="v1")
            nc.scalar.dma_start(v1_t[:], v1_b)
            m2_t = cpool.tile([128, R * 32], fp32, tag="m2")
            nc.scalar.dma_start(m2_t[:], m2_b)
            v0_t = cpool.tile([128, 32], fp32, tag="v0")
            nc.scalar.dma_start(v0_t[:], v0_b)
            w4_t = cpool.tile([128, 32], fp32, tag="w4")
            nc.scalar.dma_start(w4_t[:], w4_b)
            bbt = cpool.tile([128, 1], fp32, tag="bb")
            nc.scalar.dma_start(bbt[:], bb_t)
            b4t = cpool.tile([128, 1], fp32, tag="b4")
            nc.scalar.dma_start(b4t[:], b4_t)

            psum = ppool.tile([128, C], fp32, tag="acc")

            # init matmul: zero weights x zero rhs, start=True claims the
            # whole bank's has_written bits so all later matmuls (start=False)
            # overwrite-on-first-touch / accumulate-after, independent of
            # window interleaving.
            zw = cpool.tile([128, 128], bf16, tag="zw")
            nc.vector.memset(zw[:], 0.0)
            zr = cpool.tile([128, C], bf16, tag="zr")
            nc.vector.memset(zr[:], 0.0)
            nc.tensor.matmul(psum[:, :], zw[:], zr[:], start=True, stop=False)

            # ---- main loop ----
            for s in range(S):
                xt = xpool.tile([128, ST_ * C], f8, tag="xt")
                # alternate the two HWDGE rings so DMA fixed costs overlap
                eng = nc.sync if (s % 2 == 0) else nc.scalar
                eng.dma_start(xt[:], xcc[s])
                # one-hot assign for all ST_ tiles in one DVE op:
                # asg[p, t, j] = (iota[j] == br[p, s*ST_ + t])
                asg = apool.tile([128, ST_ * W], bf16, tag="asg")
                nc.vector.tensor_tensor(
                    asg[:].rearrange("p (t j) -> p t j", j=W),
                    iota_t[:].rearrange("p (t j) -> p t j", j=W),
                    brall[:, s * ST_ : (s + 1) * ST_]
                        .rearrange("p (t o) -> p t o", o=1)
                        .to_broadcast((128, ST_, W)),
                    op=mybir.AluOpType.is_equal,
                )
                for t in range(ST_):
                    i = s * ST_ + t
                    w = i % WPC       # window interleave: spread col-groups
                    last = i == n_tiles - 1
                    nc.tensor.matmul(
                        psum[w * W : (w + 1) * W, :],
                        asg[:, t * W : (t + 1) * W],
                        xt[:, t * C : (t + 1) * C],
                        start=False,
                        stop=last,
                        tile_position=(0, w * W),
                    )

            # ---- epilogue (per-graph folded MLP) ----
            sb = epool.tile([128, C], fp32, tag="sb")
            nc.vector.tensor_copy(sb[:], psum[:])

            mx = epool.tile([128, D], fp32, tag="mx")
            nc.vector.tensor_scalar_mul(mx[:], sb[:, 0:D], rec_t[:])
            mr = epool.tile([128, R], fp32, tag="mr")
            nc.vector.tensor_scalar_mul(mr[:], sb[:, D : D + R], rec_t[:])

            # gp = rowsum(mean_x * W_base) + b_base
            t1 = epool.tile([128, D], fp32, tag="t1")
            nc.vector.tensor_mul(t1[:], mx[:], wb_t[:])
            gp = epool.tile([128, 1], fp32, tag="gp")
            nc.vector.tensor_reduce(gp[:], t1[:], axis=mybir.AxisListType.X,
                                    op=mybir.AluOpType.add)
            nc.vector.tensor_add(gp[:], gp[:], bbt[:])

            # pre = gp*v1 + sum_j mr[:,j]*M2[j] + v0
            pre = epool.tile([128, 32], fp32, tag="pre")
            nc.vector.tensor_scalar_mul(pre[:], v1_t[:], gp[:])
            tmp = epool.tile([128, 32], fp32, tag="tmp")
            for j in range(R):
                nc.vector.tensor_scalar(
                    tmp[:], m2_t[:, j * 32 : (j + 1) * 32], mr[:, j : j + 1], None,
                    op0=mybir.AluOpType.mult,
                )
                nc.vector.tensor_add(pre[:], pre[:], tmp[:])
            nc.vector.tensor_add(pre[:], pre[:], v0_t[:])

            act = epool.tile([128, 32], fp32, tag="act")
            nc.scalar.activation(act[:], pre[:], mybir.ActivationFunctionType.Relu)

            # out = rowsum(act * W4) + b4
            nc.vector.tensor_mul(act[:], act[:], w4_t[:])
            oo = epool.tile([128, 1], fp32, tag="oo")
            nc.vector.tensor_reduce(oo[:], act[:], axis=mybir.AxisListType.X,
                                    op=mybir.AluOpType.add)
            nc.vector.tensor_add(oo[:], oo[:], b4t[:])

            nc.sync.dma_start(out_d, oo[:])

    nc.compile()
    return nc


def kernel(x, clock_period, batch, W_base, b_base, W1, b1, W2, b2, W3, b3, W4, b4,
           _profile=None):
    x = np.asarray(x, np.float32)
    clock = np.asarray(clock_period, np.float32).reshape(-1)
    batch = np.asarray(batch, np.int32)
    W_base = np.asarray(W_base, np.float32)
    W1 = np.asarray(W1, np.float32); b1 = np.asarray(b1, np.float32)
    W2 = np.asarray(W2, np.float32); b2 = np.asarray(b2, np.float32)
    W3 = np.asarray(W3, np.float32); b3 = np.asarray(b3, np.float32)
    W4 = np.asarray(W4, np.float32); b4 = np.asarray(b4, np.float32)
    hid = W1.shape[1]

    # r-path: exact algebraic fold when relu(c*W1 + b1) == c * relu(W1)
    fold = bool(np.all(b1 == 0.0)) and bool(clock.min() >= 0.0)
    if fold:
        R = 1
        r32 = clock[:, None]                                   # [N, 1]
        q = np.maximum(W1, 0.0) @ W2                           # [1, hid]
        M2 = q @ W3[1:, :]                                     # [1, 32]
        v0 = b2 @ W3[1:, :] + b3                               # [32]
    else:
        R = hid
        r32 = np.maximum(clock[:, None] @ W1 + b1, 0.0)        # [N, hid]
        M2 = W2 @ W3[1:, :]                                    # [hid, 32]
        v0 = b2 @ W3[1:, :] + b3

    C = D + R               # fp8 payload: [x | r]

    # ---- shard by graph; window padding so tile->window map is static ----
    cut = np.searchsorted(batch, np.arange(0, N_GRAPHS + 1, W))
    win_nodes = np.diff(cut)
    T_w = int(math.ceil(win_nodes.max() / 128.0))
    while (WPC * T_w) % ST:
        T_w += 1
    n_tiles = WPC * T_w
    S = n_tiles // ST

    # per-graph reciprocal counts (reference divides by max(cnt, 1))
    cnt = np.bincount(batch, minlength=N_GRAPHS).astype(np.float32)
    rec_all = (1.0 / np.maximum(cnt, 1.0)).astype(np.float32)

    pay8 = np.concatenate([x, r32], axis=1).astype(F8)         # [N, C]

    in_maps = []
    # shared constant tiles
    iota_c = np.broadcast_to(
        np.tile(np.arange(W, dtype=BF16), ST)[None, :], (128, ST * W)
    ).copy()
    wbase_b = np.broadcast_to(W_base[:, 0][None, :], (128, D)).astype(np.float32).copy()
    v1_b = np.broadcast_to(W3[0, :][None, :], (128, 32)).astype(np.float32).copy()
    m2_b = np.broadcast_to(M2.reshape(-1)[None, :], (128, R * 32)).astype(np.float32).copy()
    v0_b = np.broadcast_to(v0[None, :], (128, 32)).astype(np.float32).copy()
    w4_b = np.broadcast_to(W4[:, 0][None, :], (128, 32)).astype(np.float32).copy()
    bb_t = np.full((128, 1), float(b_base.reshape(-1)[0]), np.float32)
    b4_t = np.full((128, 1), float(b4.reshape(-1)[0]), np.float32)

    for k in range(N_CORES):
        # window blocks: [WPC, T_w*128, C] zero-padded, then interleave tiles
        # so that issue-order tile i = (window i%WPC, in-window tile i//WPC)
        wblk = np.zeros((WPC, T_w * 128, C), F8)
        brw = np.full((WPC, T_w * 128), -1.0, BF16)
        for wi in range(WPC):
            gw = k * WPC + wi          # global window index
            s0, e0 = int(cut[gw]), int(cut[gw + 1])
            n = e0 - s0
            wblk[wi, :n] = pay8[s0:e0]
            brw[wi, :n] = (batch[s0:e0] - gw * W).astype(BF16)
        # [WPC, T_w, 128, C] -> issue order [T_w, WPC, 128, C] -> [n_tiles,128,C]
        tiles = wblk.reshape(WPC, T_w, 128, C).transpose(1, 0, 2, 3) \
                    .reshape(n_tiles, 128, C)
        brt = brw.reshape(WPC, T_w, 128).transpose(1, 0, 2).reshape(n_tiles, 128)
        brs = np.ascontiguousarray(brt.T)                      # [128, n_tiles]
        # permute so each SBUF partition line is contiguous in DRAM
        xcc_p = np.ascontiguousarray(
            tiles.reshape(S, ST, 128, C).transpose(0, 2, 1, 3)
        ).reshape(S, 128, ST * C)
        rec_c = np.ascontiguousarray(
            rec_all[k * GPC : (k + 1) * GPC][:, None])         # [128, 1]
        in_maps.append(dict(
            xcc=xcc_p, brs=brs, iota_c=iota_c, rec_d=rec_c,
            wbase_b=wbase_b, v1_b=v1_b, m2_b=m2_b, v0_b=v0_b, w4_b=w4_b,
            bb_t=bb_t, b4_t=b4_t,
        ))

    nc = _build_program(S, ST, C, R)

    kw = {}
    if _profile is not None:
        kw = dict(trace=True, **_profile)
    res = run_bass_kernel_spmd(nc, in_maps, list(range(N_CORES)), **kw)

    out = np.concatenate([res.results[k]["out"] for k in range(N_CORES)], axis=0)
    if _profile is not None:
        return out.astype(np.float32), res
    return out.astype(np.float32)
